# revision 28
# baseline (speedup 1.0000x reference)
"""Trainium2 Bass kernel for nn_KANSplineLayer (KAN spline layer, 8-core SPMD).

Math rewrite (validated in the v1 baseline):
  reference: out = silu(BN_b(x @ Wb)) + BN_s(basis(minmax(x)) @ Ws.T)
  Each per-(o,i) spline is continuous piecewise-linear on z in [0,1] with
  breakpoints {0,.25,.5,.75,1}; with t = 4*z it equals a linear combination of
  {t, relu(t-1), relu(t-2), relu(t-3), 1}.  BN is folded into weights/biases.

Layout strategy (v4, 117.7us vs 172us for the v1 transpose-on-device design):
  - Host stages each row-shard of x pre-transposed: x_t [256, 4096]
    (features on partitions, two 128-row blocks).  No device transposes.
  - GEMMs run weights-stationary producing out^T [outs, rows]:
      out^T[o, r] = sum_feat W[feat, o] * plane[feat, r]
    so per-output biases are per-PARTITION and ride the ACT activation bias
    (b_base inside silu, C_s on the PSUM drain).  Host de-transposes at gather.
  - fp16 planes/weights/output (>= fp32r-HIGH accuracy, fast LDWEIGHTS,
    N=512 moving operand, DVE 2x/4x modes); rel err ~4e-4 (budget 2e-2).
  - Exact fp32 per-feature min/max chained across row chunks on DVE,
    overlapped with the input DMA.  AllGather of [min | -max] (lower floor
    than AllReduce) + one strided local min-reduce over the 8 ranks.
  - Base GEMM + SiLU run during the collective wait (they don't need
    min/max); a warmup matmul block flips the PE HAM throttle to full clock
    before they start.  Plane construction is split DVE/ACT so neither
    engine gates the spline matmul cadence.
  Residual walls (measured): ~21us fixed NEFF/cc-stream init + a 29-45us
  cc-setup barrier + ~11us ncfw latency + ~10us AllGather op for the 2KB
  stats exchange, and a firmware 13/16 clock throttle that caps the spline
  window at ~263ns/matmul.
"""
import os
import numpy as np

import concourse.bacc as bacc
import concourse.bass as bass
import concourse.tile as tile
from concourse import mybir
from concourse.bass_utils import run_bass_kernel_spmd

# ---- problem constants (hardcoded; kernel.py must be self-contained) ----
IN_F, OUT_F = 256, 256
K_KNOTS = 9
EPS_MINMAX = 1e-7
EPS_BN = 1e-3
B, H, W = 32, 32, 32
N_TOTAL = B * H * W            # 32768 rows
N_CORES = 8
N_SHARD = N_TOTAL // N_CORES   # 4096 rows per core
CH = 512                       # row chunk (moving free dim, PSUM bank width)
N_CHUNKS = N_SHARD // CH       # 8
N_MATS = 5                     # [Wb, W_t, H1, H2, H3]

F32 = mybir.dt.float32

# debug/bisect knobs (default = full-performance config)
NOJUNK = os.environ.get("KAN_NOJUNK", "0") == "1"
NOF16 = os.environ.get("KAN_NOF16", "0") == "1"
DVECAST = os.environ.get("KAN_DVECAST", "0") == "1"

F16 = mybir.dt.float32 if NOF16 else mybir.dt.float16
NP16 = np.float32 if NOF16 else np.float16


def _host_prep(base_weight, spline_weight, spline_scaler,
               bn_base_gamma, bn_base_beta, bn_base_mean, bn_base_var,
               bn_spline_gamma, bn_spline_beta, bn_spline_mean, bn_spline_var):
    """Fold BN + rewrite spline into relu-plane weights. All in float64.

    Returns:
      w_blob [128, N_MATS*2*2*128] f16 : per-partition packed stationary tiles,
             index (m, fb, ob, col) -> weight[fb*128+p, ob*128+col] of mat m
      bias   [128, 4] f32 : cols (b_b ob0, b_b ob1, C_s ob0, C_s ob1)
    """
    f64 = np.float64
    w = np.asarray(spline_weight, f64) * np.asarray(spline_scaler, f64)[:, :, None]
    knots = np.linspace(-1.0, 1.0, K_KNOTS).astype(f64)
    jg = np.arange(5, dtype=f64) / 4.0
    tri = np.maximum(0.0, 1.0 - np.abs(jg[None, :] - knots[:, None]))   # [k, j]
    G = np.einsum('oik,kj->oij', w, tri)                                # [o,i,5]
    a_s = np.asarray(bn_spline_gamma, f64) / np.sqrt(np.asarray(bn_spline_var, f64) + EPS_BN)
    b_s = np.asarray(bn_spline_beta, f64) - a_s * np.asarray(bn_spline_mean, f64)
    G = G * a_s[:, None, None]
    W_t = (G[:, :, 1] - G[:, :, 0]).T                                   # [i,o]
    H1 = (G[:, :, 2] - 2 * G[:, :, 1] + G[:, :, 0]).T
    H2 = (G[:, :, 3] - 2 * G[:, :, 2] + G[:, :, 1]).T
    H3 = (G[:, :, 4] - 2 * G[:, :, 3] + G[:, :, 2]).T
    C_s = G[:, :, 0].sum(axis=1) + b_s                                  # [o]
    a_b = np.asarray(bn_base_gamma, f64) / np.sqrt(np.asarray(bn_base_var, f64) + EPS_BN)
    b_b = np.asarray(bn_base_beta, f64) - a_b * np.asarray(bn_base_mean, f64)
    Wb = np.asarray(base_weight, f64) * a_b[None, :]                    # [i,o]

    mats = [Wb, W_t, H1, H2, H3]                                        # each [i,o]
    blob = np.empty((128, N_MATS, 2, 2, 128), np.float64)
    for m, M in enumerate(mats):
        for fb in range(2):
            for ob in range(2):
                blob[:, m, fb, ob, :] = M[fb * 128:(fb + 1) * 128,
                                          ob * 128:(ob + 1) * 128]
    w_blob = blob.reshape(128, N_MATS * 2 * 2 * 128).astype(NP16)
    bias = np.stack([b_b[0:128], b_b[128:256],
                     C_s[0:128], C_s[128:256]], axis=1).astype(np.float32)
    return w_blob, bias


def _build_bass():
    nc = bacc.Bacc(num_devices=N_CORES)
    x_t = nc.declare_dram_parameter("x_t", [IN_F, N_SHARD], F32, isOutput=False)
    w_d = nc.declare_dram_parameter("w_blob", [128, N_MATS * 2 * 2 * 128], F16,
                                    isOutput=False)
    bias_d = nc.declare_dram_parameter("bias", [128, 4], F32, isOutput=False)
    out_t = nc.declare_dram_parameter("out_t", [OUT_F, N_SHARD], F16, isOutput=True)

    from contextlib import ExitStack
    with tile.TileContext(nc) as tc, ExitStack() as es:
        cons = es.enter_context(tc.tile_pool(name="cons", bufs=1))
        xin_p = es.enter_context(tc.tile_pool(name="xin", bufs=3))
        scr_p = es.enter_context(tc.tile_pool(name="scr", bufs=4))
        planes_p = es.enter_context(tc.tile_pool(name="planes", bufs=2))
        silu_p = es.enter_context(tc.tile_pool(name="silu", bufs=1))
        spl_p = es.enter_context(tc.tile_pool(name="spl", bufs=4))
        outp = es.enter_context(tc.tile_pool(name="outp", bufs=4))
        psB = es.enter_context(tc.tile_pool(name="psB", bufs=7, space="PSUM"))
        psW = es.enter_context(tc.tile_pool(name="psW", bufs=1, space="PSUM"))
        dram = es.enter_context(tc.tile_pool(name="dram", bufs=4, space="DRAM"))

        # ---- constants ----
        wsb_flat = cons.tile([128, N_MATS * 2 * 2 * 128], F16)
        nc.sync.dma_start(out=wsb_flat[:], in_=w_d[:])
        wsb = wsb_flat[:].rearrange("p (m b o n) -> p m b o n",
                                    m=N_MATS, b=2, o=2, n=128)
        bsb = cons.tile([128, 4], F32)
        nc.sync.dma_start(out=bsb[:], in_=bias_d[:])

        # relu-plane biases for the ACT path: cols [-1, -2, -3]
        rb = cons.tile([128, 3], F32)
        for m in (1, 2, 3):
            nc.vector.memset(rb[:, m - 1:m], -float(m))

        # running per-feature stats: cols [min0, min1, max0, max1]
        mm_acc = cons.tile([128, 4], F32)
        nc.vector.memset(mm_acc[:, 0:2], 3.0e38)
        nc.vector.memset(mm_acc[:, 2:4], -3.0e38)

        # xb: fp16 copy of x^T, feature blocks on partitions
        xb = cons.tile([128, 2, N_SHARD], F16)

        cast = nc.vector.tensor_copy if DVECAST else nc.scalar.copy

        # ---- phase 1: stream x chunks; cast to fp16; chained min/max ----
        for c in range(N_CHUNKS):
            cs = slice(c * CH, (c + 1) * CH)
            xin = xin_p.tile([128, 2, CH], F32, tag="xin")
            for fb in range(2):
                nc.sync.dma_start(out=xin[:, fb, :],
                                  in_=x_t[fb * 128:(fb + 1) * 128, cs])
            for fb in range(2):
                cast(out=xb[:, fb, cs], in_=xin[:, fb, :])
            # NOTE: nc.vector.tensor_tensor_reduce would fuse min/max+combine
            # at half the DVE cycles, but it HANGS this hw/runtime (verified
            # with both chained-accum and float-init forms).  One X-axis
            # reduce covers both feature blocks at once: [128,2,512]->[128,2].
            part = scr_p.tile([128, 4], F32, tag="part")
            nc.vector.tensor_reduce(
                out=part[:, 0:2], in_=xin[:],
                op=mybir.AluOpType.min, axis=mybir.AxisListType.X)
            nc.vector.tensor_reduce(
                out=part[:, 2:4], in_=xin[:],
                op=mybir.AluOpType.max, axis=mybir.AxisListType.X)
            nc.vector.tensor_tensor(
                out=mm_acc[:, 0:2], in0=mm_acc[:, 0:2],
                in1=part[:, 0:2], op=mybir.AluOpType.min)
            nc.vector.tensor_tensor(
                out=mm_acc[:, 2:4], in0=mm_acc[:, 2:4],
                in1=part[:, 2:4], op=mybir.AluOpType.max)
        # ---- global min/max: collective over [min0, min1, -max0, -max1] ----
        mm_pack = cons.tile([128, 4], F32)
        nc.vector.tensor_copy(out=mm_pack[:, 0:2], in_=mm_acc[:, 0:2])
        nc.vector.tensor_scalar(out=mm_pack[:, 2:4], in0=mm_acc[:, 2:4],
                                scalar1=-1.0, scalar2=None,
                                op0=mybir.AluOpType.mult)
        USE_AG = os.environ.get("KAN_AR", "0") != "1"
        cc_in = dram.tile([128, 4], F32)
        nc.sync.dma_start(out=cc_in[:], in_=mm_pack[:])
        gstat = cons.tile([128, 4], F32)   # [gmin0, gmin1, -gmax0, -gmax1]
        if USE_AG:
            cc_out = dram.tile([N_CORES, 128, 4], F32)
            nc.gpsimd.collective_compute(
                "AllGather", mybir.AluOpType.bypass,
                replica_groups=[list(range(N_CORES))],
                ins=[cc_in.opt()], outs=[cc_out.opt()])
            gsb = cons.tile([128, N_CORES, 4], F32)
            nc.sync.dma_start(out=gsb[:], in_=cc_out[:].rearrange("r p c -> p r c"))
            # all four stats are min-reduced over ranks (maxes are negated)
            nc.vector.tensor_reduce(
                out=gstat[:], in_=gsb[:].rearrange("p r c -> p c r"),
                op=mybir.AluOpType.min, axis=mybir.AxisListType.X)
        else:
            cc_out = dram.tile([128, 4], F32)
            nc.gpsimd.collective_compute(
                "AllReduce", mybir.AluOpType.min,
                replica_groups=[list(range(N_CORES))],
                ins=[cc_in.opt()], outs=[cc_out.opt()])
            nc.sync.dma_start(out=gstat[:], in_=cc_out[:])

        jp = None
        if not NOJUNK:
            # PE warmup: emitted after the collective so the cc-stream setup
            # barrier is the first ready instruction on the PE queue; these
            # then run during phase 1 and flip HAM to 8/8 before the base GEMM
            for i in range(18):
                jp = psW.tile([128, 512], F32, tag="junk")
                nc.tensor.matmul(jp[:], wsb[:, 0, 0, 0, :], wsb_flat[:, 0:512],
                                 start=True, stop=True, skip_group_check=True)

        # rng = gmax - gmin;  s4 = 1/(0.25*rng + eps/4)
        nrng = cons.tile([128, 2], F32)
        qt = cons.tile([128, 2], F32)
        s4 = cons.tile([128, 2], F32)
        nc.vector.tensor_tensor(out=nrng[:], in0=gstat[:, 0:2], in1=gstat[:, 2:4],
                                op=mybir.AluOpType.add)
        nc.vector.tensor_scalar(out=qt[:], in0=nrng[:],
                                scalar1=-0.25, scalar2=EPS_MINMAX * 0.25,
                                op0=mybir.AluOpType.mult, op1=mybir.AluOpType.add)
        nc.vector.reciprocal(out=s4[:], in_=qt[:])

        # ---- base path: GEMM + silu(+b_b); independent of min/max, so the
        #      scheduler runs it during the collective ----
        silu_sb = []
        for ob in range(2):
            row = []
            for c in range(N_CHUNKS):
                cs = slice(c * CH, (c + 1) * CH)
                ps = psB.tile([128, CH], F32, tag="ps")
                for fb in range(2):
                    nc.tensor.matmul(ps[:], wsb[:, 0, fb, ob, :], xb[:, fb, cs],
                                     start=(fb == 0), stop=(fb == 1),
                                     skip_group_check=True)
                sl = silu_p.tile([128, CH], F16, tag=f"silu{ob}_{c}")
                nc.scalar.activation(out=sl[:], in_=ps[:],
                                     func=mybir.ActivationFunctionType.Silu,
                                     bias=bsb[:, ob:ob + 1], scale=1.0)
                row.append(sl)
            silu_sb.append(row)

        # ---- spline: planes (DVE) -> GEMMs -> drain(+C_s) -> +silu -> DMA ----
        for c in range(N_CHUNKS):
            cs = slice(c * CH, (c + 1) * CH)
            tpl, rpl = [], [[], [], []]
            for fb in range(2):
                t = planes_p.tile([128, CH], F16, tag=f"t{fb}")
                nc.vector.tensor_scalar(
                    out=t[:], in0=xb[:, fb, cs],
                    scalar1=gstat[:, fb:fb + 1], scalar2=s4[:, fb:fb + 1],
                    op0=mybir.AluOpType.subtract, op1=mybir.AluOpType.mult)
                tpl.append(t)
                for m in (1, 2, 3):
                    r = planes_p.tile([128, CH], F16, tag=f"r{m}{fb}")
                    if m == 1:
                        # ACT is idle here apart from drains; offload r1 so
                        # DVE stops gating the spline matmul cadence
                        nc.scalar.activation(
                            out=r[:], in_=t[:],
                            func=mybir.ActivationFunctionType.Relu,
                            bias=rb[:, m - 1:m], scale=1.0)
                    else:
                        nc.vector.tensor_scalar(
                            out=r[:], in0=t[:], scalar1=float(m), scalar2=0.0,
                            op0=mybir.AluOpType.subtract, op1=mybir.AluOpType.max)
                    rpl[m - 1].append(r)
            for ob in range(2):
                ps = psB.tile([128, CH], F32, tag="ps")
                first = True
                for m in range(4):   # mats 1..4 = W_t, H1, H2, H3
                    pl = tpl if m == 0 else rpl[m - 1]
                    for fb in range(2):
                        nc.tensor.matmul(
                            ps[:], wsb[:, m + 1, fb, ob, :], pl[fb][:],
                            start=first, stop=(m == 3 and fb == 1),
                            skip_group_check=True)
                        first = False
                spl = spl_p.tile([128, CH], F32, tag="spl")
                nc.scalar.activation(out=spl[:], in_=ps[:],
                                     func=mybir.ActivationFunctionType.Identity,
                                     bias=bsb[:, 2 + ob:3 + ob], scale=1.0)
                o = outp.tile([128, CH], F16, tag="o")
                nc.vector.tensor_tensor(out=o[:], in0=spl[:],
                                        in1=silu_sb[ob][c][:],
                                        op=mybir.AluOpType.add)
                nc.sync.dma_start(out=out_t[ob * 128:(ob + 1) * 128, cs], in_=o[:])

        if not NOJUNK:
            # consume the last junk-psum tile so nothing dangles
            jfin = cons.tile([128, 1], F32)
            nc.vector.tensor_copy(out=jfin[:], in_=jp[:, 0:1])
    nc.compile()
    return nc


_CACHE = {}


def make_in_maps(inputs):
    x = np.asarray(inputs["x"], np.float32)
    w_blob, bias = _host_prep(**{k: v for k, v in inputs.items() if k != "x"})
    xf = x.reshape(N_TOTAL, IN_F)
    maps = []
    for c in range(N_CORES):
        sh = xf[c * N_SHARD:(c + 1) * N_SHARD]          # [4096, 256]
        maps.append({"x_t": np.ascontiguousarray(sh.T),  # [256, 4096]
                     "w_blob": w_blob, "bias": bias})
    return maps


def kernel(**inputs):
    if "nc" not in _CACHE:
        _CACHE["nc"] = _build_bass()
    nc = _CACHE["nc"]
    in_maps = make_in_maps(inputs)
    res = run_bass_kernel_spmd(nc, in_maps, list(range(N_CORES)))
    shards = []
    for c in range(N_CORES):
        ot = np.asarray(res.results[c]["out_t"])         # [256, 4096]
        shards.append(ot.T)                              # [4096, 256]
    out = np.concatenate(shards, axis=0)
    return np.ascontiguousarray(out).reshape(B, H, W, OUT_F).astype(np.float32)


# revision 30
# speedup vs baseline: 1.1062x; 1.1062x over previous
"""Trainium2 Bass kernel for nn_KANSplineLayer (KAN spline layer, 8-core SPMD).

Math rewrite (validated in the v1 baseline):
  reference: out = silu(BN_b(x @ Wb)) + BN_s(basis(minmax(x)) @ Ws.T)
  Each per-(o,i) spline is continuous piecewise-linear on z in [0,1] with
  breakpoints {0,.25,.5,.75,1}; with t = 4*z it equals a linear combination of
  {t, relu(t-1), relu(t-2), relu(t-3), 1}.  BN is folded into weights/biases.

Layout strategy (measured 108.9-136.7us across runs — the spread is a random
29-48us collective-setup barrier — vs a stable 172us for the v1
transpose-on-device fp32r design):
  - Host stages each row-shard of x pre-transposed: x_t [256, 4096]
    (features on partitions, two 128-row blocks).  No device transposes.
  - GEMMs run weights-stationary producing out^T [outs, rows]:
      out^T[o, r] = sum_feat W[feat, o] * plane[feat, r]
    so per-output biases are per-PARTITION and ride the ACT activation bias
    (b_base inside silu, C_s on the PSUM drain).  Host de-transposes at gather.
  - fp16 planes/weights/output (>= fp32r-HIGH accuracy, fast LDWEIGHTS,
    N=512 moving operand, DVE 2x/4x modes); rel err ~4e-4 (budget 2e-2).
  - Exact fp32 per-feature min/max chained across row chunks on DVE,
    overlapped with the input DMA.  AllGather of [min | -max] (lower floor
    than AllReduce) + one strided local min-reduce over the 8 ranks.
  - Base GEMM + SiLU run during the collective wait (they don't need
    min/max); a warmup matmul block flips the PE HAM throttle to full clock
    before they start.  Plane construction is split DVE/ACT so neither
    engine gates the spline matmul cadence.
  Residual walls (measured): ~21us fixed NEFF/cc-stream init + a 29-45us
  cc-setup barrier + ~11us ncfw latency + ~10us AllGather op for the 2KB
  stats exchange, and a firmware 13/16 clock throttle that caps the spline
  window at ~263ns/matmul.
"""
import os
import numpy as np

import concourse.bacc as bacc
import concourse.bass as bass
import concourse.tile as tile
from concourse import mybir
from concourse.bass_utils import run_bass_kernel_spmd

# ---- problem constants (hardcoded; kernel.py must be self-contained) ----
IN_F, OUT_F = 256, 256
K_KNOTS = 9
EPS_MINMAX = 1e-7
EPS_BN = 1e-3
B, H, W = 32, 32, 32
N_TOTAL = B * H * W            # 32768 rows
N_CORES = 8
N_SHARD = N_TOTAL // N_CORES   # 4096 rows per core
CH = 512                       # row chunk (moving free dim, PSUM bank width)
N_CHUNKS = N_SHARD // CH       # 8
N_MATS = 5                     # [Wb, W_t, H1, H2, H3]

F32 = mybir.dt.float32

# debug/bisect knobs (default = full-performance config)
NOJUNK = os.environ.get("KAN_NOJUNK", "0") == "1"
NOF16 = os.environ.get("KAN_NOF16", "0") == "1"
DVECAST = os.environ.get("KAN_DVECAST", "0") == "1"

F16 = mybir.dt.float32 if NOF16 else mybir.dt.float16
NP16 = np.float32 if NOF16 else np.float16


def _host_prep(base_weight, spline_weight, spline_scaler,
               bn_base_gamma, bn_base_beta, bn_base_mean, bn_base_var,
               bn_spline_gamma, bn_spline_beta, bn_spline_mean, bn_spline_var):
    """Fold BN + rewrite spline into relu-plane weights. All in float64.

    Returns:
      w_blob [128, N_MATS*2*2*128] f16 : per-partition packed stationary tiles,
             index (m, fb, ob, col) -> weight[fb*128+p, ob*128+col] of mat m
      bias   [128, 4] f32 : cols (b_b ob0, b_b ob1, C_s ob0, C_s ob1)
    """
    f64 = np.float64
    w = np.asarray(spline_weight, f64) * np.asarray(spline_scaler, f64)[:, :, None]
    knots = np.linspace(-1.0, 1.0, K_KNOTS).astype(f64)
    jg = np.arange(5, dtype=f64) / 4.0
    tri = np.maximum(0.0, 1.0 - np.abs(jg[None, :] - knots[:, None]))   # [k, j]
    G = np.einsum('oik,kj->oij', w, tri)                                # [o,i,5]
    a_s = np.asarray(bn_spline_gamma, f64) / np.sqrt(np.asarray(bn_spline_var, f64) + EPS_BN)
    b_s = np.asarray(bn_spline_beta, f64) - a_s * np.asarray(bn_spline_mean, f64)
    G = G * a_s[:, None, None]
    W_t = (G[:, :, 1] - G[:, :, 0]).T                                   # [i,o]
    H1 = (G[:, :, 2] - 2 * G[:, :, 1] + G[:, :, 0]).T
    H2 = (G[:, :, 3] - 2 * G[:, :, 2] + G[:, :, 1]).T
    H3 = (G[:, :, 4] - 2 * G[:, :, 3] + G[:, :, 2]).T
    C_s = G[:, :, 0].sum(axis=1) + b_s                                  # [o]
    a_b = np.asarray(bn_base_gamma, f64) / np.sqrt(np.asarray(bn_base_var, f64) + EPS_BN)
    b_b = np.asarray(bn_base_beta, f64) - a_b * np.asarray(bn_base_mean, f64)
    Wb = np.asarray(base_weight, f64) * a_b[None, :]                    # [i,o]

    mats = [Wb, W_t, H1, H2, H3]                                        # each [i,o]
    blob = np.empty((128, N_MATS, 2, 2, 128), np.float64)
    for m, M in enumerate(mats):
        for fb in range(2):
            for ob in range(2):
                blob[:, m, fb, ob, :] = M[fb * 128:(fb + 1) * 128,
                                          ob * 128:(ob + 1) * 128]
    w_blob = blob.reshape(128, N_MATS * 2 * 2 * 128).astype(NP16)
    bias = np.stack([b_b[0:128], b_b[128:256],
                     C_s[0:128], C_s[128:256]], axis=1).astype(np.float32)
    return w_blob, bias


def _build_bass():
    nc = bacc.Bacc(num_devices=N_CORES)
    x_t = nc.declare_dram_parameter("x_t", [IN_F, N_SHARD], F32, isOutput=False)
    w_d = nc.declare_dram_parameter("w_blob", [128, N_MATS * 2 * 2 * 128], F16,
                                    isOutput=False)
    bias_d = nc.declare_dram_parameter("bias", [128, 4], F32, isOutput=False)
    out_t = nc.declare_dram_parameter("out_t", [OUT_F, N_SHARD], F16, isOutput=True)

    from contextlib import ExitStack
    with tile.TileContext(nc) as tc, ExitStack() as es:
        cons = es.enter_context(tc.tile_pool(name="cons", bufs=1))
        xin_p = es.enter_context(tc.tile_pool(name="xin", bufs=3))
        scr_p = es.enter_context(tc.tile_pool(name="scr", bufs=4))
        planes_p = es.enter_context(tc.tile_pool(name="planes", bufs=2))
        silu_p = es.enter_context(tc.tile_pool(name="silu", bufs=1))
        spl_p = es.enter_context(tc.tile_pool(name="spl", bufs=4))
        outp = es.enter_context(tc.tile_pool(name="outp", bufs=4))
        psB = es.enter_context(tc.tile_pool(name="psB", bufs=7, space="PSUM"))
        psW = es.enter_context(tc.tile_pool(name="psW", bufs=1, space="PSUM"))
        dram = es.enter_context(tc.tile_pool(name="dram", bufs=4, space="DRAM"))

        # ---- constants ----
        wsb_flat = cons.tile([128, N_MATS * 2 * 2 * 128], F16)
        nc.sync.dma_start(out=wsb_flat[:], in_=w_d[:])
        wsb = wsb_flat[:].rearrange("p (m b o n) -> p m b o n",
                                    m=N_MATS, b=2, o=2, n=128)
        bsb = cons.tile([128, 4], F32)
        nc.sync.dma_start(out=bsb[:], in_=bias_d[:])

        # relu-plane biases for the ACT path: cols [-1, -2, -3]
        rb = cons.tile([128, 3], F32)
        for m in (1, 2, 3):
            nc.vector.memset(rb[:, m - 1:m], -float(m))

        # running per-feature stats: cols [min0, min1, max0, max1]
        mm_acc = cons.tile([128, 4], F32)
        nc.vector.memset(mm_acc[:, 0:2], 3.0e38)
        nc.vector.memset(mm_acc[:, 2:4], -3.0e38)

        # xb: fp16 copy of x^T, feature blocks on partitions
        xb = cons.tile([128, 2, N_SHARD], F16)

        cast = nc.vector.tensor_copy if DVECAST else nc.scalar.copy

        # ---- phase 1: stream x chunks; cast to fp16; chained min/max ----
        # 1024-row chunks amortize the per-op DVE reduce overhead (the
        # collective trigger is phase-1-DVE-bound on good barrier draws)
        P1CH = 1024
        for c in range(N_SHARD // P1CH):
            cs = slice(c * P1CH, (c + 1) * P1CH)
            xin = xin_p.tile([128, 2, P1CH], F32, tag="xin")
            for fb in range(2):
                nc.sync.dma_start(out=xin[:, fb, :],
                                  in_=x_t[fb * 128:(fb + 1) * 128, cs])
            for fb in range(2):
                cast(out=xb[:, fb, cs], in_=xin[:, fb, :])
            # NOTE: nc.vector.tensor_tensor_reduce would fuse min/max+combine
            # at half the DVE cycles, but it HANGS this hw/runtime (verified
            # with both chained-accum and float-init forms).  One X-axis
            # reduce covers both feature blocks at once: [128,2,N]->[128,2].
            part = scr_p.tile([128, 4], F32, tag="part")
            nc.vector.tensor_reduce(
                out=part[:, 0:2], in_=xin[:],
                op=mybir.AluOpType.min, axis=mybir.AxisListType.X)
            nc.vector.tensor_reduce(
                out=part[:, 2:4], in_=xin[:],
                op=mybir.AluOpType.max, axis=mybir.AxisListType.X)
            nc.vector.tensor_tensor(
                out=mm_acc[:, 0:2], in0=mm_acc[:, 0:2],
                in1=part[:, 0:2], op=mybir.AluOpType.min)
            nc.vector.tensor_tensor(
                out=mm_acc[:, 2:4], in0=mm_acc[:, 2:4],
                in1=part[:, 2:4], op=mybir.AluOpType.max)
        # ---- global min/max: collective over [min0, min1, -max0, -max1] ----
        mm_pack = cons.tile([128, 4], F32)
        nc.vector.tensor_copy(out=mm_pack[:, 0:2], in_=mm_acc[:, 0:2])
        nc.vector.tensor_scalar(out=mm_pack[:, 2:4], in0=mm_acc[:, 2:4],
                                scalar1=-1.0, scalar2=None,
                                op0=mybir.AluOpType.mult)
        USE_AG = os.environ.get("KAN_AR", "0") != "1"
        cc_in = dram.tile([128, 4], F32)
        nc.sync.dma_start(out=cc_in[:], in_=mm_pack[:])
        gstat = cons.tile([128, 4], F32)   # [gmin0, gmin1, -gmax0, -gmax1]
        if USE_AG:
            cc_out = dram.tile([N_CORES, 128, 4], F32)
            nc.gpsimd.collective_compute(
                "AllGather", mybir.AluOpType.bypass,
                replica_groups=[list(range(N_CORES))],
                ins=[cc_in.opt()], outs=[cc_out.opt()])
            gsb = cons.tile([128, N_CORES, 4], F32)
            nc.sync.dma_start(out=gsb[:], in_=cc_out[:].rearrange("r p c -> p r c"))
            # all four stats are min-reduced over ranks (maxes are negated)
            nc.vector.tensor_reduce(
                out=gstat[:], in_=gsb[:].rearrange("p r c -> p c r"),
                op=mybir.AluOpType.min, axis=mybir.AxisListType.X)
        else:
            cc_out = dram.tile([128, 4], F32)
            nc.gpsimd.collective_compute(
                "AllReduce", mybir.AluOpType.min,
                replica_groups=[list(range(N_CORES))],
                ins=[cc_in.opt()], outs=[cc_out.opt()])
            nc.sync.dma_start(out=gstat[:], in_=cc_out[:])

        jp = None
        if not NOJUNK:
            # PE warmup: emitted after the collective so the cc-stream setup
            # barrier is the first ready instruction on the PE queue; these
            # then run during phase 1 and flip HAM to 8/8 before the base GEMM
            for i in range(18):
                jp = psW.tile([128, 512], F32, tag="junk")
                nc.tensor.matmul(jp[:], wsb[:, 0, 0, 0, :], wsb_flat[:, 0:512],
                                 start=True, stop=True, skip_group_check=True)

        # rng = gmax - gmin;  s4 = 1/(0.25*rng + eps/4)
        nrng = cons.tile([128, 2], F32)
        qt = cons.tile([128, 2], F32)
        s4 = cons.tile([128, 2], F32)
        nc.vector.tensor_tensor(out=nrng[:], in0=gstat[:, 0:2], in1=gstat[:, 2:4],
                                op=mybir.AluOpType.add)
        nc.vector.tensor_scalar(out=qt[:], in0=nrng[:],
                                scalar1=-0.25, scalar2=EPS_MINMAX * 0.25,
                                op0=mybir.AluOpType.mult, op1=mybir.AluOpType.add)
        nc.vector.reciprocal(out=s4[:], in_=qt[:])

        # ---- base path: GEMM + silu(+b_b); independent of min/max, so the
        #      scheduler runs it during the collective ----
        silu_sb = []
        for ob in range(2):
            row = []
            for c in range(N_CHUNKS):
                cs = slice(c * CH, (c + 1) * CH)
                ps = psB.tile([128, CH], F32, tag="ps")
                for fb in range(2):
                    nc.tensor.matmul(ps[:], wsb[:, 0, fb, ob, :], xb[:, fb, cs],
                                     start=(fb == 0), stop=(fb == 1),
                                     skip_group_check=True)
                sl = silu_p.tile([128, CH], F16, tag=f"silu{ob}_{c}")
                nc.scalar.activation(out=sl[:], in_=ps[:],
                                     func=mybir.ActivationFunctionType.Silu,
                                     bias=bsb[:, ob:ob + 1], scale=1.0)
                row.append(sl)
            silu_sb.append(row)

        # ---- spline: planes (DVE) -> GEMMs -> drain(+C_s) -> +silu -> DMA ----
        for c in range(N_CHUNKS):
            cs = slice(c * CH, (c + 1) * CH)
            tpl, rpl = [], [[], [], []]
            for fb in range(2):
                t = planes_p.tile([128, CH], F16, tag=f"t{fb}")
                nc.vector.tensor_scalar(
                    out=t[:], in0=xb[:, fb, cs],
                    scalar1=gstat[:, fb:fb + 1], scalar2=s4[:, fb:fb + 1],
                    op0=mybir.AluOpType.subtract, op1=mybir.AluOpType.mult)
                tpl.append(t)
                for m in (1, 2, 3):
                    r = planes_p.tile([128, CH], F16, tag=f"r{m}{fb}")
                    if m == 1:
                        # ACT is idle here apart from drains; offload r1 so
                        # DVE stops gating the spline matmul cadence
                        nc.scalar.activation(
                            out=r[:], in_=t[:],
                            func=mybir.ActivationFunctionType.Relu,
                            bias=rb[:, m - 1:m], scale=1.0)
                    else:
                        nc.vector.tensor_scalar(
                            out=r[:], in0=t[:], scalar1=float(m), scalar2=0.0,
                            op0=mybir.AluOpType.subtract, op1=mybir.AluOpType.max)
                    rpl[m - 1].append(r)
            for ob in range(2):
                ps = psB.tile([128, CH], F32, tag="ps")
                first = True
                for m in range(4):   # mats 1..4 = W_t, H1, H2, H3
                    pl = tpl if m == 0 else rpl[m - 1]
                    for fb in range(2):
                        nc.tensor.matmul(
                            ps[:], wsb[:, m + 1, fb, ob, :], pl[fb][:],
                            start=first, stop=(m == 3 and fb == 1),
                            skip_group_check=True)
                        first = False
                spl = spl_p.tile([128, CH], F32, tag="spl")
                nc.scalar.activation(out=spl[:], in_=ps[:],
                                     func=mybir.ActivationFunctionType.Identity,
                                     bias=bsb[:, 2 + ob:3 + ob], scale=1.0)
                o = outp.tile([128, CH], F16, tag="o")
                nc.vector.tensor_tensor(out=o[:], in0=spl[:],
                                        in1=silu_sb[ob][c][:],
                                        op=mybir.AluOpType.add)
                nc.sync.dma_start(out=out_t[ob * 128:(ob + 1) * 128, cs], in_=o[:])

        if not NOJUNK:
            # consume the last junk-psum tile so nothing dangles
            jfin = cons.tile([128, 1], F32)
            nc.vector.tensor_copy(out=jfin[:], in_=jp[:, 0:1])
    nc.compile()
    return nc


_CACHE = {}


def make_in_maps(inputs):
    x = np.asarray(inputs["x"], np.float32)
    w_blob, bias = _host_prep(**{k: v for k, v in inputs.items() if k != "x"})
    xf = x.reshape(N_TOTAL, IN_F)
    maps = []
    for c in range(N_CORES):
        sh = xf[c * N_SHARD:(c + 1) * N_SHARD]          # [4096, 256]
        maps.append({"x_t": np.ascontiguousarray(sh.T),  # [256, 4096]
                     "w_blob": w_blob, "bias": bias})
    return maps


def kernel(**inputs):
    if "nc" not in _CACHE:
        _CACHE["nc"] = _build_bass()
    nc = _CACHE["nc"]
    in_maps = make_in_maps(inputs)
    res = run_bass_kernel_spmd(nc, in_maps, list(range(N_CORES)))
    shards = []
    for c in range(N_CORES):
        ot = np.asarray(res.results[c]["out_t"])         # [256, 4096]
        shards.append(ot.T)                              # [4096, 256]
    out = np.concatenate(shards, axis=0)
    return np.ascontiguousarray(out).reshape(B, H, W, OUT_F).astype(np.float32)


# revision 31
# speedup vs baseline: 1.1329x; 1.0242x over previous
"""Trainium2 Bass kernel for nn_KANSplineLayer (KAN spline layer, 8-core SPMD).

Math rewrite (validated in the v1 baseline):
  reference: out = silu(BN_b(x @ Wb)) + BN_s(basis(minmax(x)) @ Ws.T)
  Each per-(o,i) spline is continuous piecewise-linear on z in [0,1] with
  breakpoints {0,.25,.5,.75,1}; with t = 4*z it equals a linear combination of
  {t, relu(t-1), relu(t-2), relu(t-3), 1}.  BN is folded into weights/biases.

Layout strategy (measured 108.9-136.7us across runs — the spread is a random
29-48us collective-setup barrier — vs a stable 172us for the v1
transpose-on-device fp32r design):
  - Host stages each row-shard of x pre-transposed: x_t [256, 4096]
    (features on partitions, two 128-row blocks).  No device transposes.
  - GEMMs run weights-stationary producing out^T [outs, rows]:
      out^T[o, r] = sum_feat W[feat, o] * plane[feat, r]
    so per-output biases are per-PARTITION and ride the ACT activation bias
    (b_base inside silu, C_s on the PSUM drain).  Host de-transposes at gather.
  - fp16 planes/weights/output (>= fp32r-HIGH accuracy, fast LDWEIGHTS,
    N=512 moving operand, DVE 2x/4x modes); rel err ~4e-4 (budget 2e-2).
  - Exact fp32 per-feature min/max chained across row chunks on DVE,
    overlapped with the input DMA.  AllGather of [min | -max] (lower floor
    than AllReduce) + one strided local min-reduce over the 8 ranks.
  - Base GEMM + SiLU run during the collective wait (they don't need
    min/max); a warmup matmul block flips the PE HAM throttle to full clock
    before they start.  Plane construction is split DVE/ACT so neither
    engine gates the spline matmul cadence.
  Residual walls (measured): ~21us fixed NEFF/cc-stream init + a 29-45us
  cc-setup barrier + ~11us ncfw latency + ~10us AllGather op for the 2KB
  stats exchange, and a firmware 13/16 clock throttle that caps the spline
  window at ~263ns/matmul.
"""
import os
import numpy as np

import concourse.bacc as bacc
import concourse.bass as bass
import concourse.tile as tile
from concourse import mybir
from concourse.bass_utils import run_bass_kernel_spmd

# ---- problem constants (hardcoded; kernel.py must be self-contained) ----
IN_F, OUT_F = 256, 256
K_KNOTS = 9
EPS_MINMAX = 1e-7
EPS_BN = 1e-3
B, H, W = 32, 32, 32
N_TOTAL = B * H * W            # 32768 rows
N_CORES = 8
N_SHARD = N_TOTAL // N_CORES   # 4096 rows per core
CH = 512                       # row chunk (moving free dim, PSUM bank width)
N_CHUNKS = N_SHARD // CH       # 8
N_MATS = 5                     # [Wb, W_t, H1, H2, H3]

F32 = mybir.dt.float32

# debug/bisect knobs (default = full-performance config)
NOJUNK = os.environ.get("KAN_NOJUNK", "0") == "1"
NOF16 = os.environ.get("KAN_NOF16", "0") == "1"
DVECAST = os.environ.get("KAN_DVECAST", "0") == "1"

F16 = mybir.dt.float32 if NOF16 else mybir.dt.float16
NP16 = np.float32 if NOF16 else np.float16


def _host_prep(base_weight, spline_weight, spline_scaler,
               bn_base_gamma, bn_base_beta, bn_base_mean, bn_base_var,
               bn_spline_gamma, bn_spline_beta, bn_spline_mean, bn_spline_var):
    """Fold BN + rewrite spline into relu-plane weights. All in float64.

    Returns:
      w_blob [128, N_MATS*2*2*128] f16 : per-partition packed stationary tiles,
             index (m, fb, ob, col) -> weight[fb*128+p, ob*128+col] of mat m
      bias   [128, 4] f32 : cols (b_b ob0, b_b ob1, C_s ob0, C_s ob1)
    """
    f64 = np.float64
    w = np.asarray(spline_weight, f64) * np.asarray(spline_scaler, f64)[:, :, None]
    knots = np.linspace(-1.0, 1.0, K_KNOTS).astype(f64)
    jg = np.arange(5, dtype=f64) / 4.0
    tri = np.maximum(0.0, 1.0 - np.abs(jg[None, :] - knots[:, None]))   # [k, j]
    G = np.einsum('oik,kj->oij', w, tri)                                # [o,i,5]
    a_s = np.asarray(bn_spline_gamma, f64) / np.sqrt(np.asarray(bn_spline_var, f64) + EPS_BN)
    b_s = np.asarray(bn_spline_beta, f64) - a_s * np.asarray(bn_spline_mean, f64)
    G = G * a_s[:, None, None]
    W_t = (G[:, :, 1] - G[:, :, 0]).T                                   # [i,o]
    H1 = (G[:, :, 2] - 2 * G[:, :, 1] + G[:, :, 0]).T
    H2 = (G[:, :, 3] - 2 * G[:, :, 2] + G[:, :, 1]).T
    H3 = (G[:, :, 4] - 2 * G[:, :, 3] + G[:, :, 2]).T
    C_s = G[:, :, 0].sum(axis=1) + b_s                                  # [o]
    a_b = np.asarray(bn_base_gamma, f64) / np.sqrt(np.asarray(bn_base_var, f64) + EPS_BN)
    b_b = np.asarray(bn_base_beta, f64) - a_b * np.asarray(bn_base_mean, f64)
    Wb = np.asarray(base_weight, f64) * a_b[None, :]                    # [i,o]

    mats = [Wb, W_t, H1, H2, H3]                                        # each [i,o]
    blob = np.empty((128, N_MATS, 2, 2, 128), np.float64)
    for m, M in enumerate(mats):
        for fb in range(2):
            for ob in range(2):
                blob[:, m, fb, ob, :] = M[fb * 128:(fb + 1) * 128,
                                          ob * 128:(ob + 1) * 128]
    w_blob = blob.reshape(128, N_MATS * 2 * 2 * 128).astype(NP16)
    bias = np.stack([b_b[0:128], b_b[128:256],
                     C_s[0:128], C_s[128:256]], axis=1).astype(np.float32)
    return w_blob, bias


def _build_bass():
    nc = bacc.Bacc(num_devices=N_CORES)
    x_t = nc.declare_dram_parameter("x_t", [IN_F, N_SHARD], F32, isOutput=False)
    w_d = nc.declare_dram_parameter("w_blob", [128, N_MATS * 2 * 2 * 128], F16,
                                    isOutput=False)
    bias_d = nc.declare_dram_parameter("bias", [128, 4], F32, isOutput=False)
    out_t = nc.declare_dram_parameter("out_t", [OUT_F, N_SHARD], F16, isOutput=True)

    from contextlib import ExitStack
    with tile.TileContext(nc) as tc, ExitStack() as es:
        cons = es.enter_context(tc.tile_pool(name="cons", bufs=1))
        xin_p = es.enter_context(tc.tile_pool(name="xin", bufs=3))
        scr_p = es.enter_context(tc.tile_pool(name="scr", bufs=4))
        planes_p = es.enter_context(tc.tile_pool(name="planes", bufs=2))
        silu_p = es.enter_context(tc.tile_pool(name="silu", bufs=1))
        spl_p = es.enter_context(tc.tile_pool(name="spl", bufs=4))
        outp = es.enter_context(tc.tile_pool(name="outp", bufs=4))
        psB = es.enter_context(tc.tile_pool(name="psB", bufs=7, space="PSUM"))
        psW = es.enter_context(tc.tile_pool(name="psW", bufs=1, space="PSUM"))
        dram = es.enter_context(tc.tile_pool(name="dram", bufs=4, space="DRAM"))

        # ---- constants ----
        wsb_flat = cons.tile([128, N_MATS * 2 * 2 * 128], F16)
        nc.sync.dma_start(out=wsb_flat[:], in_=w_d[:])
        wsb = wsb_flat[:].rearrange("p (m b o n) -> p m b o n",
                                    m=N_MATS, b=2, o=2, n=128)
        bsb = cons.tile([128, 4], F32)
        nc.sync.dma_start(out=bsb[:], in_=bias_d[:])

        # relu-plane biases for the ACT path: cols [-1, -2, -3]
        rb = cons.tile([128, 3], F32)
        for m in (1, 2, 3):
            nc.vector.memset(rb[:, m - 1:m], -float(m))

        # running per-feature stats: cols [min0, min1, max0, max1]
        mm_acc = cons.tile([128, 4], F32)
        nc.vector.memset(mm_acc[:, 0:2], 3.0e38)
        nc.vector.memset(mm_acc[:, 2:4], -3.0e38)

        # xb: fp16 copy of x^T, feature blocks on partitions
        xb = cons.tile([128, 2, N_SHARD], F16)

        cast = nc.vector.tensor_copy if DVECAST else nc.scalar.copy

        # ---- phase 1: stream x chunks; cast to fp16; chained min/max ----
        # 512-row chunks measured best: 1024-row chunks delay the first
        # reduce and push the collective trigger ~4us later
        P1CH = 512
        for c in range(N_SHARD // P1CH):
            cs = slice(c * P1CH, (c + 1) * P1CH)
            xin = xin_p.tile([128, 2, P1CH], F32, tag="xin")
            for fb in range(2):
                nc.sync.dma_start(out=xin[:, fb, :],
                                  in_=x_t[fb * 128:(fb + 1) * 128, cs])
            for fb in range(2):
                cast(out=xb[:, fb, cs], in_=xin[:, fb, :])
            # NOTE: nc.vector.tensor_tensor_reduce would fuse min/max+combine
            # at half the DVE cycles, but it HANGS this hw/runtime (verified
            # with both chained-accum and float-init forms).  One X-axis
            # reduce covers both feature blocks at once: [128,2,N]->[128,2].
            part = scr_p.tile([128, 4], F32, tag="part")
            nc.vector.tensor_reduce(
                out=part[:, 0:2], in_=xin[:],
                op=mybir.AluOpType.min, axis=mybir.AxisListType.X)
            nc.vector.tensor_reduce(
                out=part[:, 2:4], in_=xin[:],
                op=mybir.AluOpType.max, axis=mybir.AxisListType.X)
            nc.vector.tensor_tensor(
                out=mm_acc[:, 0:2], in0=mm_acc[:, 0:2],
                in1=part[:, 0:2], op=mybir.AluOpType.min)
            nc.vector.tensor_tensor(
                out=mm_acc[:, 2:4], in0=mm_acc[:, 2:4],
                in1=part[:, 2:4], op=mybir.AluOpType.max)
        # ---- global min/max: collective over [min0, min1, -max0, -max1] ----
        mm_pack = cons.tile([128, 4], F32)
        nc.vector.tensor_copy(out=mm_pack[:, 0:2], in_=mm_acc[:, 0:2])
        nc.vector.tensor_scalar(out=mm_pack[:, 2:4], in0=mm_acc[:, 2:4],
                                scalar1=-1.0, scalar2=None,
                                op0=mybir.AluOpType.mult)
        USE_AG = os.environ.get("KAN_AR", "0") != "1"
        cc_in = dram.tile([128, 4], F32)
        nc.sync.dma_start(out=cc_in[:], in_=mm_pack[:])
        gstat = cons.tile([128, 4], F32)   # [gmin0, gmin1, -gmax0, -gmax1]
        if USE_AG:
            cc_out = dram.tile([N_CORES, 128, 4], F32)
            nc.gpsimd.collective_compute(
                "AllGather", mybir.AluOpType.bypass,
                replica_groups=[list(range(N_CORES))],
                ins=[cc_in.opt()], outs=[cc_out.opt()])
            gsb = cons.tile([128, N_CORES, 4], F32)
            nc.sync.dma_start(out=gsb[:], in_=cc_out[:].rearrange("r p c -> p r c"))
            # all four stats are min-reduced over ranks (maxes are negated)
            nc.vector.tensor_reduce(
                out=gstat[:], in_=gsb[:].rearrange("p r c -> p c r"),
                op=mybir.AluOpType.min, axis=mybir.AxisListType.X)
        else:
            cc_out = dram.tile([128, 4], F32)
            nc.gpsimd.collective_compute(
                "AllReduce", mybir.AluOpType.min,
                replica_groups=[list(range(N_CORES))],
                ins=[cc_in.opt()], outs=[cc_out.opt()])
            nc.sync.dma_start(out=gstat[:], in_=cc_out[:])

        jp = None
        if not NOJUNK:
            # PE warmup: emitted after the collective so the cc-stream setup
            # barrier is the first ready instruction on the PE queue; these
            # then run during phase 1 and flip HAM to 8/8 before the base GEMM
            for i in range(18):
                jp = psW.tile([128, 512], F32, tag="junk")
                nc.tensor.matmul(jp[:], wsb[:, 0, 0, 0, :], wsb_flat[:, 0:512],
                                 start=True, stop=True, skip_group_check=True)

        # rng = gmax - gmin;  s4 = 1/(0.25*rng + eps/4)
        nrng = cons.tile([128, 2], F32)
        qt = cons.tile([128, 2], F32)
        s4 = cons.tile([128, 2], F32)
        nc.vector.tensor_tensor(out=nrng[:], in0=gstat[:, 0:2], in1=gstat[:, 2:4],
                                op=mybir.AluOpType.add)
        nc.vector.tensor_scalar(out=qt[:], in0=nrng[:],
                                scalar1=-0.25, scalar2=EPS_MINMAX * 0.25,
                                op0=mybir.AluOpType.mult, op1=mybir.AluOpType.add)
        nc.vector.reciprocal(out=s4[:], in_=qt[:])

        # ---- base path: GEMM + silu(+b_b); independent of min/max, so the
        #      scheduler runs it during the collective ----
        silu_sb = []
        for ob in range(2):
            row = []
            for c in range(N_CHUNKS):
                cs = slice(c * CH, (c + 1) * CH)
                ps = psB.tile([128, CH], F32, tag="ps")
                for fb in range(2):
                    nc.tensor.matmul(ps[:], wsb[:, 0, fb, ob, :], xb[:, fb, cs],
                                     start=(fb == 0), stop=(fb == 1),
                                     skip_group_check=True)
                sl = silu_p.tile([128, CH], F16, tag=f"silu{ob}_{c}")
                nc.scalar.activation(out=sl[:], in_=ps[:],
                                     func=mybir.ActivationFunctionType.Silu,
                                     bias=bsb[:, ob:ob + 1], scale=1.0)
                row.append(sl)
            silu_sb.append(row)

        # ---- spline: planes (DVE) -> GEMMs -> drain(+C_s) -> +silu -> DMA ----
        for c in range(N_CHUNKS):
            cs = slice(c * CH, (c + 1) * CH)
            tpl, rpl = [], [[], [], []]
            for fb in range(2):
                t = planes_p.tile([128, CH], F16, tag=f"t{fb}")
                nc.vector.tensor_scalar(
                    out=t[:], in0=xb[:, fb, cs],
                    scalar1=gstat[:, fb:fb + 1], scalar2=s4[:, fb:fb + 1],
                    op0=mybir.AluOpType.subtract, op1=mybir.AluOpType.mult)
                tpl.append(t)
                for m in (1, 2, 3):
                    r = planes_p.tile([128, CH], F16, tag=f"r{m}{fb}")
                    if m == 1:
                        # ACT is idle here apart from drains; offload r1 so
                        # DVE stops gating the spline matmul cadence
                        nc.scalar.activation(
                            out=r[:], in_=t[:],
                            func=mybir.ActivationFunctionType.Relu,
                            bias=rb[:, m - 1:m], scale=1.0)
                    else:
                        nc.vector.tensor_scalar(
                            out=r[:], in0=t[:], scalar1=float(m), scalar2=0.0,
                            op0=mybir.AluOpType.subtract, op1=mybir.AluOpType.max)
                    rpl[m - 1].append(r)
            for ob in range(2):
                ps = psB.tile([128, CH], F32, tag="ps")
                first = True
                for m in range(4):   # mats 1..4 = W_t, H1, H2, H3
                    pl = tpl if m == 0 else rpl[m - 1]
                    for fb in range(2):
                        nc.tensor.matmul(
                            ps[:], wsb[:, m + 1, fb, ob, :], pl[fb][:],
                            start=first, stop=(m == 3 and fb == 1),
                            skip_group_check=True)
                        first = False
                spl = spl_p.tile([128, CH], F32, tag="spl")
                nc.scalar.activation(out=spl[:], in_=ps[:],
                                     func=mybir.ActivationFunctionType.Identity,
                                     bias=bsb[:, 2 + ob:3 + ob], scale=1.0)
                o = outp.tile([128, CH], F16, tag="o")
                nc.vector.tensor_tensor(out=o[:], in0=spl[:],
                                        in1=silu_sb[ob][c][:],
                                        op=mybir.AluOpType.add)
                nc.sync.dma_start(out=out_t[ob * 128:(ob + 1) * 128, cs], in_=o[:])

        if not NOJUNK:
            # consume the last junk-psum tile so nothing dangles
            jfin = cons.tile([128, 1], F32)
            nc.vector.tensor_copy(out=jfin[:], in_=jp[:, 0:1])
    nc.compile()
    return nc


_CACHE = {}


def make_in_maps(inputs):
    x = np.asarray(inputs["x"], np.float32)
    w_blob, bias = _host_prep(**{k: v for k, v in inputs.items() if k != "x"})
    xf = x.reshape(N_TOTAL, IN_F)
    maps = []
    for c in range(N_CORES):
        sh = xf[c * N_SHARD:(c + 1) * N_SHARD]          # [4096, 256]
        maps.append({"x_t": np.ascontiguousarray(sh.T),  # [256, 4096]
                     "w_blob": w_blob, "bias": bias})
    return maps


def kernel(**inputs):
    if "nc" not in _CACHE:
        _CACHE["nc"] = _build_bass()
    nc = _CACHE["nc"]
    in_maps = make_in_maps(inputs)
    res = run_bass_kernel_spmd(nc, in_maps, list(range(N_CORES)))
    shards = []
    for c in range(N_CORES):
        ot = np.asarray(res.results[c]["out_t"])         # [256, 4096]
        shards.append(ot.T)                              # [4096, 256]
    out = np.concatenate(shards, axis=0)
    return np.ascontiguousarray(out).reshape(B, H, W, OUT_F).astype(np.float32)
